# revision 11
# baseline (speedup 1.0000x reference)
"""Trainium2 Bass kernel for DifferentiableExtrusion.

Full inputs in, full output out. Sharding: the 96x96=9216 grid points are
split across 8 cores (12 grid rows / 1152 points each). Every core processes
all valid polygons (host-compacted, per-batch padded) against its points:

  per (point, edge):  d^2 = l^2 + r^2   with
      l = v . n_hat               (line distance, affine in point -> PE matmul)
      u = v . e / sqrt(e^2+eps)   (affine in point -> PE matmul)
      r = u - clip(u, 0, S)       (projection excess)
  inside test: ray-cast parity of [(sign(py-y0) != sign(py-y1)) & (G > 0)]
      with G = inter_x - px       (affine in point -> PE matmul)
  sdf = sign * sqrt(min_edges d^2), per-batch min over polys, sigmoid(-100*sdf),
  depth extrusion via K=1 outer-product matmul with the depth mask.

Each core writes out[b, d, its 12 rows] = [4, 96, 1152]; host concatenates.
"""

import numpy as np

VOX = 96
SHARP = 100.0
EPS = 1e-8
NCORES = 8
M = VOX * VOX
MP = M // NCORES          # 1152 points per core
CHUNKS = MP // 128        # 9
PEDGES = 32               # edges per polygon
BLK = 512                 # matmul / elementwise block width (columns)

# matmul dtype for the three affine column groups ("float32" safe,
# "float32r" ~4x faster on PE with ~2^-15 product error)
MM_DTYPE = "float32"


def _host_prep(polygons, attributes, validity_scores):
    B, N, P, _ = polygons.shape
    assert P == PEDGES
    valid = np.asarray(validity_scores) >= 0.5
    counts = valid.sum(1)
    NP = max(1, int(counts.max()))
    NPT = B * NP
    E = NPT * P

    v0 = np.asarray(polygons, np.float32).astype(np.float64)
    v1 = np.roll(v0, -1, axis=2)
    x0, y0 = v0[..., 0], v0[..., 1]
    x1, y1 = v1[..., 0], v1[..., 1]
    ex, ey = x1 - x0, y1 - y0
    esq = ex * ex + ey * ey
    esq_c = np.maximum(esq, 1e-12)          # guard (no degenerate edges in practice)
    Sp = np.sqrt(esq + EPS)
    rt = np.sqrt(esq_c)
    s = ex / (ey + EPS)

    # coefficient tables [B,N,P,3] for features (px, py, 1)
    cu = np.stack([ex / Sp, ey / Sp, -(x0 * ex + y0 * ey) / Sp], -1)
    cl = np.stack([-ey / rt, ex / rt, (ey * x0 - ex * y0) / rt], -1)
    cg = np.stack([-np.ones_like(s), s, x0 - s * y0], -1)

    wu = np.zeros((3, E), np.float32)
    wl = np.zeros((3, E), np.float32)
    wg = np.zeros((3, E), np.float32)
    y0r = np.full(E, 5.0, np.float32)
    y1r = np.full(E, 5.0, np.float32)
    sr = np.ones(E, np.float32)
    # dummy defaults: u=0, l=1e3, G=-1 -> d2=1e6, outside
    wl[2, :] = 1e3
    wg[2, :] = -1.0

    for b in range(B):
        idx = np.nonzero(valid[b])[0]
        for k, n in enumerate(idx):
            c0 = (b * NP + k) * P
            sl = slice(c0, c0 + P)
            wu[:, sl] = cu[b, n].T.astype(np.float32)
            wl[:, sl] = cl[b, n].T.astype(np.float32)
            wg[:, sl] = cg[b, n].T.astype(np.float32)
            y0r[sl] = y0[b, n].astype(np.float32)
            y1r[sl] = y1[b, n].astype(np.float32)
            sr[sl] = Sp[b, n].astype(np.float32)

    # grid features per core
    ygrid, xgrid = np.meshgrid(np.arange(VOX, dtype=np.float32),
                               np.arange(VOX, dtype=np.float32), indexing="ij")
    px = (xgrid.ravel() / np.float32(VOX - 1)).astype(np.float32)
    py = (ygrid.ravel() / np.float32(VOX - 1)).astype(np.float32)

    feats, pybs = [], []
    for k in range(NCORES):
        sl = slice(k * MP, (k + 1) * MP)
        f = np.stack([px[sl], py[sl], np.ones(MP, np.float32)], 0)
        feats.append(np.ascontiguousarray(f, np.float32))
        pybs.append(np.ascontiguousarray(
            py[sl].reshape(CHUNKS, 128).T, np.float32))     # [128, CHUNKS]

    # depth extrusion mask  [1, B*VOX]  (single partition row for K=1 matmuls)
    attr = np.asarray(attributes, np.float32)
    norm_h = np.clip(attr[:, 0], 0.0, 1.0)
    hv = np.clip(np.round(norm_h * VOX), 1.0, float(VOX)).astype(np.float32)
    dm = (np.arange(VOX, dtype=np.float32)[None, :] < hv[:, None]).astype(np.float32)
    dmr = np.ascontiguousarray(dm.reshape(1, B * VOX))

    ident = np.eye(128, dtype=np.float32)
    rep = np.ones((128, 1), np.float32)
    tables = {
        "wu": wu, "wl": wl, "wg": wg,
        "y0b": np.ascontiguousarray(rep * y0r[None, :]),
        "y1b": np.ascontiguousarray(rep * y1r[None, :]),
        "sb": np.ascontiguousarray(rep * sr[None, :]),
        "dmr": dmr, "ident": ident,
    }
    return tables, feats, pybs, NP, E


def _build(B, NP, E):
    import concourse.bass as bass
    import concourse.tile as tile
    from concourse import bacc, mybir

    f32 = mybir.dt.float32
    mmdt = getattr(mybir.dt, MM_DTYPE)

    def mm(ap):
        return ap if MM_DTYPE == "float32" else ap.bitcast(mmdt)

    Op = mybir.AluOpType
    Act = mybir.ActivationFunctionType
    NPT = B * NP
    NBLK = (E + BLK - 1) // BLK

    nc = bacc.Bacc("TRN2", target_bir_lowering=False, debug=False)

    din = {}
    for name, shape in [("wu", [3, E]), ("wl", [3, E]), ("wg", [3, E]),
                        ("y0b", [128, E]), ("y1b", [128, E]), ("sb", [128, E]),
                        ("feat", [3, MP]), ("pyb", [128, CHUNKS]),
                        ("dmr", [1, B * VOX]), ("ident", [128, 128])]:
        din[name] = nc.dram_tensor(name, shape, f32, kind="ExternalInput")
    out_d = nc.dram_tensor("out", [B, VOX, MP], f32, kind="ExternalOutput")

    with tile.TileContext(nc) as tc:
        with tc.tile_pool(name="const", bufs=1) as cpool, \
             tc.tile_pool(name="work", bufs=3) as wpool, \
             tc.tile_pool(name="acc", bufs=2) as apool, \
             tc.tile_pool(name="psum", bufs=2, space="PSUM") as ppool, \
             tc.tile_pool(name="pout", bufs=2, space="PSUM") as opool:

            sb = {}
            for name in ["wu", "wl", "wg"]:
                t = cpool.tile([3, E], f32, tag=f"c_{name}", name=f"c_{name}")
                nc.sync.dma_start(t[:], din[name][:])
                sb[name] = t
            for name in ["y0b", "y1b", "sb"]:
                t = cpool.tile([128, E], f32, tag=f"c_{name}", name=f"c_{name}")
                for j in range(NBLK):
                    c = slice(j * BLK, min(E, (j + 1) * BLK))
                    nc.sync.dma_start(t[:, c], din[name][:, c])
                sb[name] = t
            feat = cpool.tile([3, MP], f32)
            nc.sync.dma_start(feat[:], din["feat"][:])
            pyb = cpool.tile([128, CHUNKS], f32)
            nc.sync.dma_start(pyb[:], din["pyb"][:])
            dmr = cpool.tile([1, B * VOX], f32)
            nc.sync.dma_start(dmr[:], din["dmr"][:])
            ident = cpool.tile([128, 128], f32)
            nc.sync.dma_start(ident[:], din["ident"][:])
            comb = []
            for b in range(B):
                comb_b = cpool.tile([1, MP], f32, tag=f"comb{b}", name=f"comb{b}")
                comb.append(comb_b)

            for c in range(CHUNKS):
                featc = feat[:, c * 128:(c + 1) * 128]
                mind2 = apool.tile([128, NPT], f32, tag="mind2")
                cnt = apool.tile([128, NPT], f32, tag="cnt")
                for j in range(NBLK):
                    j0 = j * BLK
                    nb = min(BLK, E - j0)
                    npj = nb // PEDGES
                    cols = slice(j0, j0 + nb)
                    ups = ppool.tile([128, nb], f32, tag="u")
                    lps = ppool.tile([128, nb], f32, tag="l")
                    gps = ppool.tile([128, nb], f32, tag="g")
                    nc.tensor.matmul(ups[:], mm(featc), mm(sb["wu"][:, cols]))
                    nc.tensor.matmul(lps[:], mm(featc), mm(sb["wl"][:, cols]))
                    nc.tensor.matmul(gps[:], mm(featc), mm(sb["wg"][:, cols]))

                    m = wpool.tile([128, nb], f32, tag="m")
                    nc.vector.scalar_tensor_tensor(
                        m[:], ups[:], 0.0, sb["sb"][:, cols],
                        op0=Op.max, op1=Op.min)
                    r = wpool.tile([128, nb], f32, tag="r")
                    nc.vector.tensor_tensor(r[:], ups[:], m[:], op=Op.subtract)
                    lsq = wpool.tile([128, nb], f32, tag="lsq")
                    nc.scalar.activation(lsq[:], lps[:], Act.Square)
                    rsq = wpool.tile([128, nb], f32, tag="rsq")
                    nc.scalar.activation(rsq[:], r[:], Act.Square)
                    d2 = wpool.tile([128, nb], f32, tag="d2")
                    nc.gpsimd.tensor_tensor(d2[:], lsq[:], rsq[:], op=Op.add)

                    s0 = wpool.tile([128, nb], f32, tag="s0")
                    nc.scalar.activation(s0[:], sb["y0b"][:, cols], Act.Sign,
                                         bias=pyb[:, c:c + 1], scale=-1.0)
                    s1 = wpool.tile([128, nb], f32, tag="s1")
                    nc.scalar.activation(s1[:], sb["y1b"][:, cols], Act.Sign,
                                         bias=pyb[:, c:c + 1], scale=-1.0)
                    u1 = wpool.tile([128, nb], f32, tag="u1")
                    nc.gpsimd.tensor_tensor(u1[:], s0[:], s1[:], op=Op.subtract)
                    ysq = wpool.tile([128, nb], f32, tag="ysq")
                    nc.scalar.activation(ysq[:], u1[:], Act.Square, scale=0.5)
                    g01 = wpool.tile([128, nb], f32, tag="g01")
                    nc.vector.tensor_scalar(g01[:], gps[:], 0.0, None, op0=Op.is_gt)
                    cross = wpool.tile([128, nb], f32, tag="cross")
                    nc.gpsimd.tensor_tensor(cross[:], ysq[:], g01[:], op=Op.mult)

                    pj = slice(j0 // PEDGES, j0 // PEDGES + npj)
                    nc.vector.tensor_reduce(
                        mind2[:, pj],
                        d2[:].rearrange("p (a b) -> p a b", b=PEDGES),
                        axis=mybir.AxisListType.X, op=Op.min)
                    nc.vector.tensor_reduce(
                        cnt[:, pj],
                        cross[:].rearrange("p (a b) -> p a b", b=PEDGES),
                        axis=mybir.AxisListType.X, op=Op.add)

                cnt_i = wpool.tile([128, NPT], mybir.dt.int32, tag="cnt_i")
                nc.vector.tensor_copy(cnt_i[:], cnt[:])
                odd_i = wpool.tile([128, NPT], mybir.dt.int32, tag="odd_i")
                nc.vector.tensor_scalar(odd_i[:], cnt_i[:], 1, None,
                                        op0=Op.bitwise_and)
                sgn = wpool.tile([128, NPT], f32, tag="sgn")
                nc.vector.tensor_scalar(sgn[:], odd_i[:], -2.0, 1.0,
                                        op0=Op.mult, op1=Op.add)
                dist = wpool.tile([128, NPT], f32, tag="dist")
                nc.scalar.activation(dist[:], mind2[:], Act.Sqrt)
                sdf = wpool.tile([128, NPT], f32, tag="sdf")
                nc.vector.tensor_tensor(sdf[:], dist[:], sgn[:], op=Op.mult)
                sdfb = wpool.tile([128, B], f32, tag="sdfb")
                nc.vector.tensor_reduce(
                    sdfb[:], sdf[:].rearrange("p (b n) -> p b n", n=NP),
                    axis=mybir.AxisListType.X, op=Op.min)
                for b in range(B):
                    pst = opool.tile([1, 128], f32, tag="pp", name=f"pst{b}")
                    nc.tensor.transpose(pst[:], sdfb[:, b:b + 1], ident[:])
                    nc.scalar.activation(comb[b][:, c * 128:(c + 1) * 128],
                                         pst[:], Act.Sigmoid, scale=-SHARP)

            # depth extrusion: out[b, :, :] = dm[b] (x) combined[b]   (K=1 matmul)
            SEG = 384
            for b in range(B):
                for g in range(MP // SEG):
                    seg = slice(g * SEG, (g + 1) * SEG)
                    po = opool.tile([VOX, SEG], f32, tag="pp")
                    nc.tensor.matmul(po[:], dmr[:, b * VOX:(b + 1) * VOX],
                                     comb[b][:, seg])
                    osb = wpool.tile([VOX, SEG], f32, tag="osb")
                    nc.scalar.activation(osb[:], po[:], Act.Copy)
                    nc.sync.dma_start(out_d[b, :, seg], osb[:])

    nc.compile()
    return nc


def kernel(polygons, attributes, validity_scores):
    from concourse.bass_utils import run_bass_kernel_spmd

    B = polygons.shape[0]
    tables, feats, pybs, NP, E = _host_prep(polygons, attributes, validity_scores)
    nc = _build(B, NP, E)
    in_maps = [dict(tables, feat=feats[k], pyb=pybs[k]) for k in range(NCORES)]
    res = run_bass_kernel_spmd(nc, in_maps, list(range(NCORES))).results
    parts = [res[k]["out"].reshape(B, VOX, VOX // NCORES, VOX)
             for k in range(NCORES)]
    return np.ascontiguousarray(np.concatenate(parts, axis=2), np.float32)


# revision 15
# speedup vs baseline: 1.5379x; 1.5379x over previous
"""Trainium2 Bass kernel for DifferentiableExtrusion.

Full inputs in, full output out. Sharding: the 96x96=9216 grid points are
split across 8 cores (12 grid rows / 1152 points each). Every core processes
all valid polygons (host-compacted, variable count per batch) against its
points:

  per (point, edge):  d^2 = l^2 + r^2   with
      l = v . n_hat               (line distance, affine in point -> PE matmul)
      u = v . e / sqrt(e^2+eps)   (affine in point -> PE matmul)
      r = u - clip(u, 0, S)       (projection excess)
  inside test: ray-cast parity of [(sign(py-y0) != sign(py-y1)) & (G > 0)]
      with G = inter_x - px       (affine in point -> PE matmul)
  The y-comparisons depend only on the point's grid row: computed once per
  core at [12, E] and DMA-broadcast across partitions per chunk.
  sdf = sign * sqrt(min_edges d^2); per-batch min over polys taken on
  sign*(d^2) (order-equivalent); sqrt+sigmoid deferred to one end stage so
  the ACT engine stays on a single function table during the main loop.
  Depth extrusion = K=1 outer-product matmul with the depth mask.

Each core writes out[b, d, its 12 rows] = [4, 96, 1152]; host concatenates.
"""

import numpy as np

VOX = 96
SHARP = 100.0
EPS = 1e-8
NCORES = 8
M = VOX * VOX
MP = M // NCORES          # 1152 points per core
CHUNKS = MP // 128        # 9
ROWS = MP // VOX          # 12 grid rows per core
PEDGES = 32               # edges per polygon
BIGD = 1e3                # far-outside distance for dummy (empty-batch) polys

# matmul dtype: "float32r" is ~4x faster on PE (~2^-15 product error, validated
# against the reference); "float32" is the bit-exact fallback.
MM_DTYPE = "float32r"


def _host_prep(polygons, attributes, validity_scores):
    B, N, P, _ = polygons.shape
    assert P == PEDGES
    valid = np.asarray(validity_scores) >= 0.5
    counts = [max(1, int(v.sum())) for v in valid]   # >=1: empty batch gets a dummy
    offs = np.cumsum([0] + counts)
    NPT = int(offs[-1])
    E = NPT * P

    v0 = np.asarray(polygons, np.float32).astype(np.float64)
    v1 = np.roll(v0, -1, axis=2)
    x0, y0 = v0[..., 0], v0[..., 1]
    x1, y1 = v1[..., 0], v1[..., 1]
    ex, ey = x1 - x0, y1 - y0
    esq = ex * ex + ey * ey
    esq_c = np.maximum(esq, 1e-12)
    Sp = np.sqrt(esq + EPS)
    rt = np.sqrt(esq_c)
    s = ex / (ey + EPS)

    cu = np.stack([ex / Sp, ey / Sp, -(x0 * ex + y0 * ey) / Sp], -1)
    cl = np.stack([-ey / rt, ex / rt, (ey * x0 - ex * y0) / rt], -1)
    cg = np.stack([-np.ones_like(s), s, x0 - s * y0], -1)

    wu = np.zeros((3, E), np.float32)
    wl = np.zeros((3, E), np.float32)
    wg = np.zeros((3, E), np.float32)
    y0r = np.full(E, 5.0, np.float32)
    y1r = np.full(E, 5.0, np.float32)
    sr = np.ones(E, np.float32)
    wl[2, :] = BIGD          # dummy cols: u=0, l=BIGD, G=-1 -> far outside
    wg[2, :] = -1.0

    for b in range(B):
        idx = np.nonzero(valid[b])[0]
        for k, n in enumerate(idx):
            c0 = (offs[b] + k) * P
            sl = slice(c0, c0 + P)
            wu[:, sl] = cu[b, n].T.astype(np.float32)
            wl[:, sl] = cl[b, n].T.astype(np.float32)
            wg[:, sl] = cg[b, n].T.astype(np.float32)
            y0r[sl] = y0[b, n].astype(np.float32)
            y1r[sl] = y1[b, n].astype(np.float32)
            sr[sl] = Sp[b, n].astype(np.float32)

    ygrid, xgrid = np.meshgrid(np.arange(VOX, dtype=np.float32),
                               np.arange(VOX, dtype=np.float32), indexing="ij")
    px = (xgrid.ravel() / np.float32(VOX - 1)).astype(np.float32)
    py = (ygrid.ravel() / np.float32(VOX - 1)).astype(np.float32)

    feats, ysqbs, cntbs = [], [], []
    for k in range(NCORES):
        sl = slice(k * MP, (k + 1) * MP)
        f = np.stack([px[sl], py[sl], np.ones(MP, np.float32)], 0)
        feats.append(np.ascontiguousarray(f, np.float32))
        # ysq[row, e] = (y0<=py) xor (y1<=py), per grid row of this core,
        # expanded to the per-chunk partition->row broadcast pattern
        rows = (np.arange(ROWS, dtype=np.float32) + k * ROWS) / np.float32(VOX - 1)
        t0c = (y0r[None, :] <= rows[:, None])
        t1c = (y1r[None, :] <= rows[:, None])
        ysq12 = (t0c ^ t1c).astype(np.float32)            # [ROWS, E]
        rowidx = (np.arange(MP) // VOX).astype(np.int64)  # local row per point
        ysqbs.append(np.ascontiguousarray(
            ysq12[rowidx].reshape(CHUNKS, 128, E)))
        # per-(row, poly) active-edge counts: cnt = sum ysq*sign(G) + cntb
        cb12 = ysq12.reshape(ROWS, NPT, PEDGES).sum(2)    # [ROWS, NPT]
        cntbs.append(np.ascontiguousarray(
            cb12[rowidx].reshape(CHUNKS, 128, NPT)))

    attr = np.asarray(attributes, np.float32)
    norm_h = np.clip(attr[:, 0], 0.0, 1.0)
    hv = np.clip(np.round(norm_h * VOX), 1.0, float(VOX)).astype(np.float32)
    dm = (np.arange(VOX, dtype=np.float32)[None, :] < hv[:, None]).astype(np.float32)
    for b in range(B):
        if not valid[b].any():       # empty batch must output zeros everywhere
            dm[b] = 0.0
    dmr = np.ascontiguousarray(dm.reshape(1, B * VOX))

    tables = {
        "wu": wu, "wl": wl, "wg": wg,
        "sbc": np.ascontiguousarray(np.ones((128, 1), np.float32) * sr[None, :]),
        "dmr": dmr, "ident": np.eye(128, dtype=np.float32),
    }
    return tables, feats, ysqbs, cntbs, counts, E


def _blocks(E):
    nblk = (E + 511) // 512
    per = -(-E // (32 * nblk)) * 32           # even-ish blocks, multiple of 32
    out = []
    o = 0
    while o < E:
        nb = min(per, E - o)
        out.append((o, nb))
        o += nb
    return out


def _build(B, counts, E):
    import concourse.bass as bass
    import concourse.tile as tile
    from concourse import bacc, mybir

    f32 = mybir.dt.float32
    i32 = mybir.dt.int32
    mmdt = getattr(mybir.dt, MM_DTYPE)

    Op = mybir.AluOpType
    Act = mybir.ActivationFunctionType
    X = mybir.AxisListType.X
    NPT = sum(counts)
    offs = np.cumsum([0] + list(counts))
    blocks = _blocks(E)

    nc = bacc.Bacc("TRN2", target_bir_lowering=False, debug=False)

    din = {}
    for name, shape in [("wu", [3, E]), ("wl", [3, E]), ("wg", [3, E]),
                        ("sbc", [128, E]), ("feat", [3, MP]),
                        ("ysqb_all", [CHUNKS, 128, E]),
                        ("cntb_all", [CHUNKS, 128, NPT]),
                        ("dmr", [1, B * VOX]), ("ident", [128, 128])]:
        dt = mmdt if name in ("wu", "wl", "wg", "feat") else f32
        din[name] = nc.dram_tensor(name, shape, dt, kind="ExternalInput")
    out_d = nc.dram_tensor("out", [B, VOX, MP], f32, kind="ExternalOutput")

    with tile.TileContext(nc) as tc:
        with tc.tile_pool(name="const", bufs=1) as cpool, \
             tc.tile_pool(name="work", bufs=3) as wpool, \
             tc.tile_pool(name="ybc", bufs=2) as ypool, \
             tc.tile_pool(name="acc", bufs=2) as apool, \
             tc.tile_pool(name="psum", bufs=2, space="PSUM") as ppool, \
             tc.tile_pool(name="pout", bufs=2, space="PSUM") as opool:

            sb = {}
            for name in ["wu", "wl", "wg"]:
                t = cpool.tile([3, E], mmdt, tag=f"c_{name}", name=f"c_{name}")
                nc.sync.dma_start(t[:], din[name][:])
                sb[name] = t
            sbc = cpool.tile([128, E], f32)
            for j0, nb in blocks:
                nc.sync.dma_start(sbc[:, j0:j0 + nb], din["sbc"][:, j0:j0 + nb])
            feat = cpool.tile([3, MP], mmdt)
            nc.sync.dma_start(feat[:], din["feat"][:])
            dmr = cpool.tile([1, B * VOX], f32)
            nc.sync.dma_start(dmr[:], din["dmr"][:])
            ident = cpool.tile([128, 128], f32)
            nc.sync.dma_start(ident[:], din["ident"][:])
            cntb = cpool.tile([128, CHUNKS, NPT], f32)
            for c in range(CHUNKS):
                nc.sync.dma_start(cntb[:, c, :], din["cntb_all"][c])

            qall = cpool.tile([128, CHUNKS * B], f32)
            comb = []
            for b in range(B):
                comb_b = cpool.tile([1, MP], f32, tag=f"comb{b}", name=f"comb{b}")
                comb.append(comb_b)

            for c in range(CHUNKS):
                featc = feat[:, c * 128:(c + 1) * 128]
                ysqb = ypool.tile([128, E], f32, tag="ysqb", name="ysqb")
                nc.sync.dma_start(ysqb[:], din["ysqb_all"][c])

                mind2 = apool.tile([128, NPT], f32, tag="mind2")
                cnt = apool.tile([128, NPT], f32, tag="cnt")
                for j0, nb in blocks:
                    npj = nb // PEDGES
                    cols = slice(j0, j0 + nb)
                    ups = ppool.tile([128, nb], f32, tag="u")
                    lps = ppool.tile([128, nb], f32, tag="l")
                    gps = ppool.tile([128, nb], f32, tag="g")
                    nc.tensor.matmul(ups[:], featc, sb["wu"][:, cols])
                    nc.tensor.matmul(lps[:], featc, sb["wl"][:, cols])
                    nc.tensor.matmul(gps[:], featc, sb["wg"][:, cols])

                    m = wpool.tile([128, nb], f32, tag="m")
                    nc.vector.scalar_tensor_tensor(
                        m[:], ups[:], 0.0, sbc[:, cols], op0=Op.max, op1=Op.min)
                    r = wpool.tile([128, nb], f32, tag="r")
                    nc.vector.tensor_tensor(r[:], ups[:], m[:], op=Op.subtract)
                    lsq = wpool.tile([128, nb], f32, tag="lsq")
                    nc.scalar.activation(lsq[:], lps[:], Act.Square)
                    rsq = wpool.tile([128, nb], f32, tag="rsq")
                    nc.scalar.activation(rsq[:], r[:], Act.Square)
                    d2 = wpool.tile([128, nb], f32, tag="d2")
                    nc.gpsimd.tensor_tensor(d2[:], lsq[:], rsq[:], op=Op.add)

                    gs = wpool.tile([128, nb], f32, tag="gs")
                    nc.scalar.activation(gs[:], gps[:], Act.Sign)
                    cross = wpool.tile([128, nb], f32, tag="cross")
                    nc.gpsimd.tensor_tensor(cross[:], gs[:], ysqb[:, cols],
                                            op=Op.mult)

                    pj = slice(j0 // PEDGES, j0 // PEDGES + npj)
                    nc.vector.tensor_reduce(
                        mind2[:, pj],
                        d2[:].rearrange("p (a b) -> p a b", b=PEDGES),
                        axis=X, op=Op.min)
                    nc.vector.tensor_reduce(
                        cnt[:, pj],
                        cross[:].rearrange("p (a b) -> p a b", b=PEDGES),
                        axis=X, op=Op.add)

                # cnt + cntb = 2*crossings (exact ints); parity from bit 1
                cnt2 = wpool.tile([128, NPT], f32, tag="cnt2")
                nc.vector.tensor_tensor(cnt2[:], cnt[:], cntb[:, c, :],
                                        op=Op.add)
                cnt_i = wpool.tile([128, NPT], i32, tag="cnt_i")
                nc.vector.tensor_copy(cnt_i[:], cnt2[:])
                odd2 = wpool.tile([128, NPT], i32, tag="odd2")
                nc.vector.tensor_scalar(odd2[:], cnt_i[:], 2, None,
                                        op0=Op.bitwise_and)
                sgn = wpool.tile([128, NPT], f32, tag="sgn")
                nc.vector.tensor_scalar(sgn[:], odd2[:], -1.0, 1.0,
                                        op0=Op.mult, op1=Op.add)
                q = wpool.tile([128, NPT], f32, tag="q")
                nc.vector.tensor_tensor(q[:], mind2[:], sgn[:], op=Op.mult)
                for b in range(B):
                    nc.vector.tensor_reduce(
                        qall[:, c * B + b:c * B + b + 1],
                        q[:, offs[b]:offs[b + 1]], axis=X, op=Op.min)

            # end stage: sdf = sign(q)*sqrt(|q|), transpose, sigmoid per batch
            absq = wpool.tile([128, CHUNKS * B], f32, tag="absq")
            nc.scalar.activation(absq[:], qall[:], Act.Abs)
            dst = wpool.tile([128, CHUNKS * B], f32, tag="dst")
            nc.scalar.activation(dst[:], absq[:], Act.Sqrt)
            sgq = wpool.tile([128, CHUNKS * B], f32, tag="sgq")
            nc.scalar.activation(sgq[:], qall[:], Act.Sign)
            sdf = wpool.tile([128, CHUNKS * B], f32, tag="sdf")
            nc.vector.tensor_tensor(sdf[:], dst[:], sgq[:], op=Op.mult)
            for c in range(CHUNKS):
                for b in range(B):
                    pst = opool.tile([1, 128], f32, tag="pp", name="pst")
                    nc.tensor.transpose(pst[:], sdf[:, c * B + b:c * B + b + 1],
                                        ident[:])
                    nc.scalar.activation(comb[b][:, c * 128:(c + 1) * 128],
                                         pst[:], Act.Sigmoid, scale=-SHARP)

            # depth extrusion: out[b] = dm[b] (x) combined[b]  (K=1 matmul)
            SEG = 384
            for b in range(B):
                for g in range(MP // SEG):
                    seg = slice(g * SEG, (g + 1) * SEG)
                    po = opool.tile([VOX, SEG], f32, tag="pp", name="po")
                    nc.tensor.matmul(po[:], dmr[:, b * VOX:(b + 1) * VOX],
                                     comb[b][:, seg])
                    osb = wpool.tile([VOX, SEG], f32, tag="osb")
                    nc.scalar.activation(osb[:], po[:], Act.Copy)
                    nc.sync.dma_start(out_d[b, :, seg], osb[:])

    nc.compile()
    return nc


def kernel(polygons, attributes, validity_scores):
    from concourse.bass_utils import run_bass_kernel_spmd

    B = polygons.shape[0]
    tables, feats, ysqbs, cntbs, counts, E = _host_prep(
        polygons, attributes, validity_scores)
    nc = _build(B, counts, E)
    in_maps = [dict(tables, feat=feats[k], ysqb_all=ysqbs[k], cntb_all=cntbs[k])
               for k in range(NCORES)]
    res = run_bass_kernel_spmd(nc, in_maps, list(range(NCORES))).results
    parts = [res[k]["out"].reshape(B, VOX, VOX // NCORES, VOX)
             for k in range(NCORES)]
    return np.ascontiguousarray(np.concatenate(parts, axis=2), np.float32)
